# revision 7
# baseline (speedup 1.0000x reference)
"""GCN encoder (2-layer, out-degree normalized) on 8 Trainium2 NeuronCores.

Strategy: shard dst nodes across cores (12544/core). Host does index-only prep:
sort edges by (dst-window, src-bank) for aggregation, by src-window for degree.
Device: deg via one-hot matmul over src-sorted tiles; h1=(x@W1)*dinv per slice;
AllGather fp16 h1 table; dma_gather (4 swdge queues) fetches per-edge messages;
one-hot compare (vector) + PE matmul accumulate per 256-node dst window;
relu+bias on PSUM evict; layer 2 same with zero-padded fp16 h2 table; out2T
slices assembled and transposed on host.
"""
import numpy as np
from contextlib import ExitStack

import concourse.bass as bass
import concourse.tile as tile
from concourse import bacc, mybir, library_config
from concourse.bass_utils import run_bass_kernel_spmd

P = 128
N = 100000
E = 1600000
IN_C, HID_C, OUT_C = 128, 128, 64
NCORE = 8
NPAD = 100352            # 8 * 12544
SLICE = NPAD // NCORE    # 12544
WAGG = 256               # agg dst-window (nodes)
NWIN = SLICE // WAGG     # 49 agg windows per core
WDEG = 128               # deg src-window
NDWIN = SLICE // WDEG    # 98 deg windows per core
NBANK = 4
BANK = NPAD // NBANK     # 25088 (< 32768, int16-safe)

TRACE = False            # test.py sets True for profiling
LAST_EXEC_NS = None
LAST_SCOPES = None


def _roundup(a, m):
    return (a + m - 1) // m * m


def _wrap16(flat_idx):
    """dma_gather idx layout: [128, n/16], wrapped by 16, replicated 8x."""
    n = flat_idx.shape[0]
    assert n % 16 == 0
    blk = flat_idx.reshape(n // 16, 16).T.astype(np.int16)   # [16, n//16]
    return np.tile(blk, (8, 1))                              # [128, n//16]


def _build_structure(src, dst):
    """Host index prep. Returns (schedule, per-core metadata arrays)."""
    src = src.astype(np.int64)
    dst = dst.astype(np.int64)

    # ---- aggregation: group edges by (global dst-window, src bank) ----
    wglobal = dst // WAGG                    # [E] in [0, 392)
    bank = src // BANK                       # [E] in [0, 4)
    segkey = wglobal * NBANK + bank          # [E]
    order = np.argsort(segkey, kind="stable")
    seg_counts = np.bincount(segkey, minlength=(NPAD // WAGG) * NBANK)
    seg_counts = seg_counts.reshape(NCORE, NWIN, NBANK)
    # uniform schedule: per (window,bank) call length = max over cores, 128-mult
    call_len = _roundup(seg_counts.max(axis=0), 128)         # [NWIN, NBANK]
    seg_starts = np.zeros((NCORE, NWIN, NBANK), dtype=np.int64)
    cum = np.cumsum(np.bincount(segkey, minlength=(NPAD // WAGG) * NBANK))
    seg_starts_flat = cum - np.bincount(segkey, minlength=(NPAD // WAGG) * NBANK)
    seg_starts = seg_starts_flat.reshape(NCORE, NWIN, NBANK)

    slots_per_win = call_len.sum(axis=1)                     # [NWIN]
    tiles_per_win = slots_per_win // 128                     # [NWIN]
    total_slots = int(slots_per_win.sum())
    total_tiles = total_slots // 128

    # slot offsets of each (w, b) call within the per-core slot space
    call_off = np.zeros((NWIN, NBANK), dtype=np.int64)
    cur = 0
    for w in range(NWIN):
        for b in range(NBANK):
            call_off[w, b] = cur
            cur += call_len[w, b]

    agg_idx = np.zeros((NCORE, total_slots), dtype=np.int16)
    agg_dst = np.full((NCORE, total_slots), -1.0, dtype=np.float32)
    src_sorted = src[order]
    dst_sorted = dst[order]
    for k in range(NCORE):
        for w in range(NWIN):
            for b in range(NBANK):
                cnt = seg_counts[k, w, b]
                s0 = seg_starts[k, w, b]
                o0 = call_off[w, b]
                agg_idx[k, o0:o0 + cnt] = (src_sorted[s0:s0 + cnt] - b * BANK)
                agg_dst[k, o0:o0 + cnt] = (
                    dst_sorted[s0:s0 + cnt] - (k * SLICE + w * WAGG)
                ).astype(np.float32)

    # idx tensor [128, total_slots//16]; dstrel tensor [128, total_tiles]
    agg_idx16 = np.stack([_wrap16(agg_idx[k]) for k in range(NCORE)])
    agg_dstrel = np.stack(
        [agg_dst[k].reshape(total_tiles, 128).T for k in range(NCORE)]
    )  # [NCORE, 128, total_tiles]

    # ---- degree: group edges by global src-window of 128 ----
    dwin = src // WDEG                       # [E] in [0, 784)
    dorder = np.argsort(dwin, kind="stable")
    dcounts = np.bincount(dwin, minlength=NPAD // WDEG).reshape(NCORE, NDWIN)
    dlen = _roundup(dcounts.max(axis=0), 128)                # [NDWIN]
    dtiles_per_win = dlen // 128
    dtotal_slots = int(dlen.sum())
    dtotal_tiles = dtotal_slots // 128
    doff = np.concatenate([[0], np.cumsum(dlen)[:-1]])
    dstarts_flat = np.concatenate(
        [[0], np.cumsum(np.bincount(dwin, minlength=NPAD // WDEG))[:-1]]
    ).reshape(NCORE, NDWIN)
    src_dsorted = src[dorder]
    deg_src = np.full((NCORE, dtotal_slots), -1.0, dtype=np.float32)
    for k in range(NCORE):
        for w in range(NDWIN):
            cnt = dcounts[k, w]
            s0 = dstarts_flat[k, w]
            o0 = doff[w]
            deg_src[k, o0:o0 + cnt] = (
                src_dsorted[s0:s0 + cnt] - (k * SLICE + w * WDEG)
            ).astype(np.float32)
    deg_srcrel = np.stack(
        [deg_src[k].reshape(dtotal_tiles, 128).T for k in range(NCORE)]
    )

    sched = {
        "call_len": call_len, "call_off": call_off,
        "tiles_per_win": tiles_per_win, "total_slots": total_slots,
        "total_tiles": total_tiles,
        "dlen": dlen, "dtiles_per_win": dtiles_per_win,
        "dtotal_tiles": dtotal_tiles,
    }
    return sched, agg_idx16, agg_dstrel, deg_srcrel


def _build_bass(sched):
    call_len = sched["call_len"]
    call_off = sched["call_off"]
    tiles_per_win = sched["tiles_per_win"]
    total_slots = sched["total_slots"]
    total_tiles = sched["total_tiles"]
    dtiles_per_win = sched["dtiles_per_win"]
    dtotal_tiles = sched["dtotal_tiles"]

    f32, f16, i16 = mybir.dt.float32, mybir.dt.float16, mybir.dt.int16
    nc = bacc.Bacc("TRN2", target_bir_lowering=False, debug=False,
                   num_devices=NCORE, num_swdge_queues=4)

    t_xT = nc.dram_tensor("xT", [P, SLICE], f32, kind="ExternalInput")
    t_W1 = nc.dram_tensor("W1", [IN_C, HID_C], f32, kind="ExternalInput")
    t_W2 = nc.dram_tensor("W2h", [HID_C, OUT_C], f16, kind="ExternalInput")
    t_b1 = nc.dram_tensor("b1c", [P, 1], f32, kind="ExternalInput")
    t_b2 = nc.dram_tensor("b2c", [OUT_C, 1], f32, kind="ExternalInput")
    t_aggidx = nc.dram_tensor("aggidx", [P, total_slots // 16], i16,
                              kind="ExternalInput")
    t_aggdst = nc.dram_tensor("aggdst", [P, total_tiles], f32,
                              kind="ExternalInput")
    t_degsrc = nc.dram_tensor("degsrc", [P, dtotal_tiles], f32,
                              kind="ExternalInput")
    t_iota = nc.dram_tensor("iotaf", [P, WAGG], f32, kind="ExternalInput")

    t_out = nc.dram_tensor("o2T", [OUT_C, SLICE], f32, kind="ExternalOutput")

    cc1_in = nc.dram_tensor("cc1_in", [SLICE, HID_C], f16, kind="Internal")
    cc1_out = nc.dram_tensor("cc1_out", [NPAD, HID_C], f16, kind="Internal",
                             addr_space="Shared")
    cc2_in = nc.dram_tensor("cc2_in", [SLICE, P], f16, kind="Internal")
    cc2_out = nc.dram_tensor("cc2_out", [NPAD, P], f16, kind="Internal",
                             addr_space="Shared")

    with tile.TileContext(nc) as tc, ExitStack() as ctx:
        const = ctx.enter_context(tc.tile_pool(name="const", bufs=1))
        meta = ctx.enter_context(tc.tile_pool(name="meta", bufs=1))
        xp = ctx.enter_context(tc.tile_pool(name="xp", bufs=4))
        hp = ctx.enter_context(tc.tile_pool(name="hp", bufs=4))
        win = ctx.enter_context(tc.tile_pool(name="win", bufs=3))
        dp = ctx.enter_context(tc.tile_pool(name="dp", bufs=6))
        ev = ctx.enter_context(tc.tile_pool(name="ev", bufs=4))
        psum = ctx.enter_context(tc.tile_pool(name="psum", bufs=4, space="PSUM"))
        psd = ctx.enter_context(tc.tile_pool(name="psd", bufs=3, space="PSUM"))

        nc.gpsimd.load_library(library_config.mlp)

        # constants / metadata loads
        W1_t = const.tile([IN_C, HID_C], f32)
        nc.sync.dma_start(W1_t[:], t_W1[:])
        W2_t = const.tile([HID_C, OUT_C], f16)
        nc.sync.dma_start(W2_t[:], t_W2[:])
        b1_t = const.tile([P, 1], f32)
        nc.sync.dma_start(b1_t[:], t_b1[:])
        b2_t = const.tile([OUT_C, 1], f32)
        nc.sync.dma_start(b2_t[:], t_b2[:])
        iota_t = const.tile([P, WAGG], f32)
        nc.sync.dma_start(iota_t[:], t_iota[:])
        ones_t = const.tile([P, 1], f16)
        nc.vector.memset(ones_t[:], 1.0)

        aggidx_t = meta.tile([P, total_slots // 16], i16)
        nc.sync.dma_start(aggidx_t[:], t_aggidx[:])
        aggdst_t = meta.tile([P, total_tiles], f32)
        nc.sync.dma_start(aggdst_t[:], t_aggdst[:])
        degsrc_t = meta.tile([P, dtotal_tiles], f32)
        nc.sync.dma_start(degsrc_t[:], t_degsrc[:])

        # ---- phase 0: degree (one-hot matmul over src-sorted tiles) ----
        # ones as stationary (loaded once) -> deg lands as rows [1, 128];
        # transpose back to per-partition columns with tiny matmuls.
        deg_row = const.tile([1, NDWIN * WDEG], f32)
        ones1_t = const.tile([1, 1], f32)
        nc.vector.memset(ones1_t[:], 1.0)
        deg_t = const.tile([P, NDWIN], f32)
        dt_idx = 0
        for w in range(NDWIN):
            pt = psd.tile([1, WDEG], f32, tag="pacc")
            nt = dtiles_per_win[w]
            for t in range(nt):
                S = dp.tile([P, WDEG], f16, tag="S")
                nc.vector.tensor_scalar(
                    out=S[:], in0=iota_t[:, 0:WDEG],
                    scalar1=degsrc_t[:, dt_idx:dt_idx + 1], scalar2=None,
                    op0=mybir.AluOpType.is_equal,
                )
                nc.tensor.matmul(pt[:], lhsT=ones_t[:], rhs=S[:],
                                 start=(t == 0), stop=(t == nt - 1))
                dt_idx += 1
            nc.scalar.copy(deg_row[:, w * WDEG:(w + 1) * WDEG], pt[:])
        for w in range(NDWIN):
            ptt = psd.tile([P, 1], f32, tag="pacc")
            nc.tensor.matmul(ptt[:], lhsT=deg_row[:, w * WDEG:(w + 1) * WDEG],
                             rhs=ones1_t[:], start=True, stop=True)
            nc.scalar.copy(deg_t[:, w:w + 1], ptt[:])
        dinv_t = const.tile([P, NDWIN], f32)
        nc.vector.tensor_scalar_max(dinv_t[:], deg_t[:], 1.0)
        nc.vector.reciprocal(dinv_t[:], dinv_t[:])

        # ---- phase 1: h1 = (x @ W1) * dinv, per 128-node block ----
        for w in range(NDWIN):
            xt = xp.tile([P, P], f32, tag="xt")
            nc.sync.dma_start(xt[:], t_xT[:, w * P:(w + 1) * P])
            ph = psd.tile([P, HID_C], f32, tag="pacc")
            nc.tensor.matmul(ph[:], lhsT=xt[:], rhs=W1_t[:], start=True,
                             stop=True)
            h1t = hp.tile([P, HID_C], f16, tag="h1t")
            nc.scalar.activation(h1t[:], ph[:],
                                 mybir.ActivationFunctionType.Copy,
                                 scale=dinv_t[:, w:w + 1])
            nc.sync.dma_start(cc1_in[w * P:(w + 1) * P, :], h1t[:])

        tc.strict_bb_all_engine_barrier()
        nc.gpsimd.collective_compute(
            "AllGather", mybir.AluOpType.bypass,
            replica_groups=[list(range(NCORE))],
            ins=[cc1_in[:]], outs=[cc1_out[:]],
        )
        tc.strict_bb_all_engine_barrier()

        # ---- phase 2: layer-1 gather + aggregate ----
        out1T = const.tile([HID_C, SLICE], f16)
        qn = 0
        for w in range(NWIN):
            nt = int(tiles_per_win[w])
            wt = win.tile([P, nt, HID_C], f16, tag="wt")
            for b in range(NBANK):
                ln = int(call_len[w, b])
                off = int(call_off[w, b])
                blk0 = (off - int(call_off[w, 0])) // 128
                nc.gpsimd.dma_gather(
                    out_ap=wt[:, blk0:blk0 + ln // 128, :],
                    in_ap=cc1_out[b * BANK:(b + 1) * BANK, :],
                    idxs_ap=aggidx_t[:, off // 16:(off + ln) // 16],
                    num_idxs=ln, num_idxs_reg=ln, elem_size=HID_C,
                    single_packet=False, queue_num=qn % 4,
                )
                qn += 1
            pw = psum.tile([HID_C, WAGG], f32, tag="aggacc")
            tbase = int(call_off[w, 0]) // 128
            for t in range(nt):
                D = dp.tile([P, WAGG], f16, tag="D")
                nc.vector.tensor_scalar(
                    out=D[:], in0=iota_t[:],
                    scalar1=aggdst_t[:, tbase + t:tbase + t + 1], scalar2=None,
                    op0=mybir.AluOpType.is_equal,
                )
                nc.tensor.matmul(pw[:], lhsT=wt[:, t, :], rhs=D[:],
                                 start=(t == 0), stop=(t == nt - 1))
            nc.scalar.activation(out1T[:, w * WAGG:(w + 1) * WAGG], pw[:],
                                 mybir.ActivationFunctionType.Relu,
                                 bias=b1_t[:])

        # ---- phase 3: h2 = (out1 @ W2) * dinv -> zero-padded fp16 table ----
        for w in range(NDWIN):
            ph = psd.tile([P, OUT_C], f32, tag="pacc")
            nc.tensor.matmul(ph[:], lhsT=out1T[:, w * P:(w + 1) * P],
                             rhs=W2_t[:], start=True, stop=True)
            h2t = hp.tile([P, P], f16, tag="h2t")
            nc.vector.memset(h2t[:, OUT_C:P], 0.0)
            nc.scalar.activation(h2t[:, 0:OUT_C], ph[:],
                                 mybir.ActivationFunctionType.Copy,
                                 scale=dinv_t[:, w:w + 1])
            nc.sync.dma_start(cc2_in[w * P:(w + 1) * P, :], h2t[:])

        tc.strict_bb_all_engine_barrier()
        nc.gpsimd.collective_compute(
            "AllGather", mybir.AluOpType.bypass,
            replica_groups=[list(range(NCORE))],
            ins=[cc2_in[:]], outs=[cc2_out[:]],
        )
        tc.strict_bb_all_engine_barrier()

        # ---- phase 4: layer-2 gather + aggregate ----
        for w in range(NWIN):
            nt = int(tiles_per_win[w])
            wt = win.tile([P, nt, P], f16, tag="wt")
            for b in range(NBANK):
                ln = int(call_len[w, b])
                off = int(call_off[w, b])
                blk0 = (off - int(call_off[w, 0])) // 128
                nc.gpsimd.dma_gather(
                    out_ap=wt[:, blk0:blk0 + ln // 128, :],
                    in_ap=cc2_out[b * BANK:(b + 1) * BANK, :],
                    idxs_ap=aggidx_t[:, off // 16:(off + ln) // 16],
                    num_idxs=ln, num_idxs_reg=ln, elem_size=P,
                    single_packet=False, queue_num=qn % 4,
                )
                qn += 1
            pw = psum.tile([P, WAGG], f32, tag="aggacc")
            tbase = int(call_off[w, 0]) // 128
            for t in range(nt):
                D = dp.tile([P, WAGG], f16, tag="D")
                nc.vector.tensor_scalar(
                    out=D[:], in0=iota_t[:],
                    scalar1=aggdst_t[:, tbase + t:tbase + t + 1], scalar2=None,
                    op0=mybir.AluOpType.is_equal,
                )
                nc.tensor.matmul(pw[:], lhsT=wt[:, t, :], rhs=D[:],
                                 start=(t == 0), stop=(t == nt - 1))
            o2 = ev.tile([OUT_C, WAGG], f32, tag="o2")
            nc.scalar.activation(o2[:], pw[0:OUT_C, :],
                                 mybir.ActivationFunctionType.Identity,
                                 bias=b2_t[:])
            nc.sync.dma_start(t_out[:, w * WAGG:(w + 1) * WAGG], o2[:])

    nc.compile()
    return nc


def kernel(x, edge_index, W1, b1, W2, b2):
    global LAST_EXEC_NS, LAST_SCOPES
    x = np.asarray(x, dtype=np.float32)
    edge_index = np.asarray(edge_index)
    W1 = np.asarray(W1, dtype=np.float32)
    b1 = np.asarray(b1, dtype=np.float32)
    W2 = np.asarray(W2, dtype=np.float32)
    b2 = np.asarray(b2, dtype=np.float32)
    src, dst = edge_index[0], edge_index[1]

    sched, agg_idx16, agg_dstrel, deg_srcrel = _build_structure(src, dst)
    nc = _build_bass(sched)

    xT = np.zeros((P, NPAD), dtype=np.float32)
    xT[:, :N] = x.T
    iota = np.broadcast_to(np.arange(WAGG, dtype=np.float32), (P, WAGG)).copy()
    b1c = np.ascontiguousarray(b1.reshape(P, 1).astype(np.float32))
    b2c = np.ascontiguousarray(b2.reshape(OUT_C, 1).astype(np.float32))
    W2h = np.ascontiguousarray(W2.astype(np.float16))

    in_maps = []
    for k in range(NCORE):
        in_maps.append({
            "xT": np.ascontiguousarray(xT[:, k * SLICE:(k + 1) * SLICE]),
            "W1": np.ascontiguousarray(W1),
            "W2h": W2h,
            "b1c": b1c,
            "b2c": b2c,
            "aggidx": np.ascontiguousarray(agg_idx16[k]),
            "aggdst": np.ascontiguousarray(agg_dstrel[k]),
            "degsrc": np.ascontiguousarray(deg_srcrel[k]),
            "iotaf": iota,
        })

    res = run_bass_kernel_spmd(nc, in_maps, core_ids=list(range(NCORE)),
                               trace=TRACE)
    LAST_EXEC_NS = res.exec_time_ns
    LAST_SCOPES = res.per_core_scope_times

    o2T = np.concatenate([res.results[k]["o2T"] for k in range(NCORE)], axis=1)
    return np.ascontiguousarray(o2T.T[:N]).astype(np.float32)


# revision 8
# speedup vs baseline: 1.0818x; 1.0818x over previous
"""GCN encoder (2-layer, out-degree normalized) on 8 Trainium2 NeuronCores.

Strategy: shard dst nodes across cores (12544/core). Host does index-only prep:
sort edges by (dst-window, src-bank) for aggregation, by src-window for degree.
Device: deg via one-hot matmul over src-sorted tiles; h1=(x@W1)*dinv per slice;
AllGather fp16 h1 table; dma_gather (4 swdge queues) fetches per-edge messages;
one-hot compare (vector) + PE matmul accumulate per 256-node dst window;
relu+bias on PSUM evict; layer 2 same with zero-padded fp16 h2 table; out2T
slices assembled and transposed on host.
"""
import numpy as np
from contextlib import ExitStack

import concourse.bass as bass
import concourse.tile as tile
from concourse import bacc, mybir, library_config
from concourse.bass_utils import run_bass_kernel_spmd

P = 128
N = 100000
E = 1600000
IN_C, HID_C, OUT_C = 128, 128, 64
NCORE = 8
NPAD = 100352            # 8 * 12544
SLICE = NPAD // NCORE    # 12544
WAGG = 256               # agg dst-window (nodes)
NWIN = SLICE // WAGG     # 49 agg windows per core
WDEG = 128               # deg src-window
NDWIN = SLICE // WDEG    # 98 deg windows per core
NBANK = 4
BANK = NPAD // NBANK     # 25088 (< 32768, int16-safe)

TRACE = False            # test.py sets True for profiling
LAST_EXEC_NS = None
LAST_SCOPES = None


def _roundup(a, m):
    return (a + m - 1) // m * m


def _wrap16(flat_idx):
    """dma_gather idx layout: [128, n/16], wrapped by 16, replicated 8x."""
    n = flat_idx.shape[0]
    assert n % 16 == 0
    blk = flat_idx.reshape(n // 16, 16).T.astype(np.int16)   # [16, n//16]
    return np.tile(blk, (8, 1))                              # [128, n//16]


def _build_structure(src, dst):
    """Host index prep. Returns (schedule, per-core metadata arrays)."""
    src = src.astype(np.int64)
    dst = dst.astype(np.int64)

    # ---- aggregation: group edges by (global dst-window, src bank) ----
    wglobal = dst // WAGG                    # [E] in [0, 392)
    bank = src // BANK                       # [E] in [0, 4)
    segkey = wglobal * NBANK + bank          # [E]
    order = np.argsort(segkey, kind="stable")
    seg_counts = np.bincount(segkey, minlength=(NPAD // WAGG) * NBANK)
    seg_counts = seg_counts.reshape(NCORE, NWIN, NBANK)
    # uniform schedule: per (window,bank) call length = max over cores, 128-mult
    call_len = _roundup(seg_counts.max(axis=0), 128)         # [NWIN, NBANK]
    seg_starts = np.zeros((NCORE, NWIN, NBANK), dtype=np.int64)
    cum = np.cumsum(np.bincount(segkey, minlength=(NPAD // WAGG) * NBANK))
    seg_starts_flat = cum - np.bincount(segkey, minlength=(NPAD // WAGG) * NBANK)
    seg_starts = seg_starts_flat.reshape(NCORE, NWIN, NBANK)

    slots_per_win = call_len.sum(axis=1)                     # [NWIN]
    tiles_per_win = slots_per_win // 128                     # [NWIN]
    total_slots = int(slots_per_win.sum())
    total_tiles = total_slots // 128

    # slot offsets of each (w, b) call within the per-core slot space
    call_off = np.zeros((NWIN, NBANK), dtype=np.int64)
    cur = 0
    for w in range(NWIN):
        for b in range(NBANK):
            call_off[w, b] = cur
            cur += call_len[w, b]

    agg_idx = np.zeros((NCORE, total_slots), dtype=np.int16)
    agg_dst = np.full((NCORE, total_slots), -1.0, dtype=np.float32)
    src_sorted = src[order]
    dst_sorted = dst[order]
    for k in range(NCORE):
        for w in range(NWIN):
            for b in range(NBANK):
                cnt = seg_counts[k, w, b]
                s0 = seg_starts[k, w, b]
                o0 = call_off[w, b]
                agg_idx[k, o0:o0 + cnt] = (src_sorted[s0:s0 + cnt] - b * BANK)
                agg_dst[k, o0:o0 + cnt] = (
                    dst_sorted[s0:s0 + cnt] - (k * SLICE + w * WAGG)
                ).astype(np.float32)

    # idx tensor [128, total_slots//16]; dstrel tensor [128, total_tiles]
    agg_idx16 = np.stack([_wrap16(agg_idx[k]) for k in range(NCORE)])
    agg_dstrel = np.stack(
        [agg_dst[k].reshape(total_tiles, 128).T for k in range(NCORE)]
    )  # [NCORE, 128, total_tiles]

    # ---- degree: group edges by global src-window of 128 ----
    dwin = src // WDEG                       # [E] in [0, 784)
    dorder = np.argsort(dwin, kind="stable")
    dcounts = np.bincount(dwin, minlength=NPAD // WDEG).reshape(NCORE, NDWIN)
    dlen = _roundup(dcounts.max(axis=0), 128)                # [NDWIN]
    dtiles_per_win = dlen // 128
    dtotal_slots = int(dlen.sum())
    dtotal_tiles = dtotal_slots // 128
    doff = np.concatenate([[0], np.cumsum(dlen)[:-1]])
    dstarts_flat = np.concatenate(
        [[0], np.cumsum(np.bincount(dwin, minlength=NPAD // WDEG))[:-1]]
    ).reshape(NCORE, NDWIN)
    src_dsorted = src[dorder]
    deg_src = np.full((NCORE, dtotal_slots), -1.0, dtype=np.float32)
    for k in range(NCORE):
        for w in range(NDWIN):
            cnt = dcounts[k, w]
            s0 = dstarts_flat[k, w]
            o0 = doff[w]
            deg_src[k, o0:o0 + cnt] = (
                src_dsorted[s0:s0 + cnt] - (k * SLICE + w * WDEG)
            ).astype(np.float32)
    deg_srcrel = np.stack(
        [deg_src[k].reshape(dtotal_tiles, 128).T for k in range(NCORE)]
    )

    sched = {
        "call_len": call_len, "call_off": call_off,
        "tiles_per_win": tiles_per_win, "total_slots": total_slots,
        "total_tiles": total_tiles,
        "dlen": dlen, "dtiles_per_win": dtiles_per_win,
        "dtotal_tiles": dtotal_tiles,
    }
    return sched, agg_idx16, agg_dstrel, deg_srcrel


def _build_bass(sched):
    call_len = sched["call_len"]
    call_off = sched["call_off"]
    tiles_per_win = sched["tiles_per_win"]
    total_slots = sched["total_slots"]
    total_tiles = sched["total_tiles"]
    dtiles_per_win = sched["dtiles_per_win"]
    dtotal_tiles = sched["dtotal_tiles"]

    f32, f16, i16 = mybir.dt.float32, mybir.dt.float16, mybir.dt.int16
    nc = bacc.Bacc("TRN2", target_bir_lowering=False, debug=False,
                   num_devices=NCORE, num_swdge_queues=4)

    t_xT = nc.dram_tensor("xT", [P, SLICE], f32, kind="ExternalInput")
    t_W1 = nc.dram_tensor("W1", [IN_C, HID_C], f32, kind="ExternalInput")
    t_W2 = nc.dram_tensor("W2h", [HID_C, OUT_C], f16, kind="ExternalInput")
    t_b1 = nc.dram_tensor("b1c", [P, 1], f32, kind="ExternalInput")
    t_b2 = nc.dram_tensor("b2c", [OUT_C, 1], f32, kind="ExternalInput")
    t_aggidx = nc.dram_tensor("aggidx", [P, total_slots // 16], i16,
                              kind="ExternalInput")
    t_aggdst = nc.dram_tensor("aggdst", [P, total_tiles], f32,
                              kind="ExternalInput")
    t_degsrc = nc.dram_tensor("degsrc", [P, dtotal_tiles], f32,
                              kind="ExternalInput")
    t_iota = nc.dram_tensor("iotaf", [P, WAGG], f16, kind="ExternalInput")

    t_out = nc.dram_tensor("o2T", [OUT_C, SLICE], f32, kind="ExternalOutput")

    cc1_in = nc.dram_tensor("cc1_in", [SLICE, HID_C], f16, kind="Internal")
    cc1_out = nc.dram_tensor("cc1_out", [NPAD, HID_C], f16, kind="Internal",
                             addr_space="Shared")
    cc2_in = nc.dram_tensor("cc2_in", [SLICE, P], f16, kind="Internal")
    cc2_out = nc.dram_tensor("cc2_out", [NPAD, P], f16, kind="Internal",
                             addr_space="Shared")

    with tile.TileContext(nc) as tc, ExitStack() as ctx:
        const = ctx.enter_context(tc.tile_pool(name="const", bufs=1))
        meta = ctx.enter_context(tc.tile_pool(name="meta", bufs=1))
        xp = ctx.enter_context(tc.tile_pool(name="xp", bufs=4))
        hp = ctx.enter_context(tc.tile_pool(name="hp", bufs=4))
        win = ctx.enter_context(tc.tile_pool(name="win", bufs=3))
        dp = ctx.enter_context(tc.tile_pool(name="dp", bufs=6))
        ev = ctx.enter_context(tc.tile_pool(name="ev", bufs=4))
        psum = ctx.enter_context(tc.tile_pool(name="psum", bufs=4, space="PSUM"))
        psd = ctx.enter_context(tc.tile_pool(name="psd", bufs=3, space="PSUM"))

        nc.gpsimd.load_library(library_config.mlp)

        # constants / metadata loads
        W1_t = const.tile([IN_C, HID_C], f32)
        nc.sync.dma_start(W1_t[:], t_W1[:])
        W2_t = const.tile([HID_C, OUT_C], f16)
        nc.sync.dma_start(W2_t[:], t_W2[:])
        b1_t = const.tile([P, 1], f32)
        nc.sync.dma_start(b1_t[:], t_b1[:])
        b2_t = const.tile([OUT_C, 1], f32)
        nc.sync.dma_start(b2_t[:], t_b2[:])
        iota_t = const.tile([P, WAGG], f16)
        nc.sync.dma_start(iota_t[:], t_iota[:])
        ones_t = const.tile([P, 1], f16)
        nc.vector.memset(ones_t[:], 1.0)

        aggidx_t = meta.tile([P, total_slots // 16], i16)
        nc.sync.dma_start(aggidx_t[:], t_aggidx[:])
        aggdst_t = meta.tile([P, total_tiles], f32)
        nc.sync.dma_start(aggdst_t[:], t_aggdst[:])
        degsrc_t = meta.tile([P, dtotal_tiles], f32)
        nc.sync.dma_start(degsrc_t[:], t_degsrc[:])

        # ---- phase 0: degree (one-hot matmul over src-sorted tiles) ----
        # ones as stationary (loaded once) -> deg lands as rows [1, 128];
        # transpose back to per-partition columns with tiny matmuls.
        deg_row = const.tile([1, NDWIN * WDEG], f32)
        ones1_t = const.tile([1, 1], f32)
        nc.vector.memset(ones1_t[:], 1.0)
        deg_t = const.tile([P, NDWIN], f32)
        dt_idx = 0
        for w in range(NDWIN):
            pt = psd.tile([1, WDEG], f32, tag="pacc")
            nt = dtiles_per_win[w]
            for t in range(nt):
                S = dp.tile([P, WDEG], f16, tag="S")
                nc.vector.tensor_scalar(
                    out=S[:], in0=iota_t[:, 0:WDEG],
                    scalar1=degsrc_t[:, dt_idx:dt_idx + 1], scalar2=None,
                    op0=mybir.AluOpType.is_equal,
                )
                nc.tensor.matmul(pt[:], lhsT=ones_t[:], rhs=S[:],
                                 start=(t == 0), stop=(t == nt - 1))
                dt_idx += 1
            nc.scalar.copy(deg_row[:, w * WDEG:(w + 1) * WDEG], pt[:])
        for w in range(NDWIN):
            ptt = psd.tile([P, 1], f32, tag="pacc")
            nc.tensor.matmul(ptt[:], lhsT=deg_row[:, w * WDEG:(w + 1) * WDEG],
                             rhs=ones1_t[:], start=True, stop=True)
            nc.scalar.copy(deg_t[:, w:w + 1], ptt[:])
        dinv_t = const.tile([P, NDWIN], f32)
        nc.vector.tensor_scalar_max(dinv_t[:], deg_t[:], 1.0)
        nc.vector.reciprocal(dinv_t[:], dinv_t[:])

        # ---- phase 1: h1 = (x @ W1) * dinv, per 128-node block ----
        for w in range(NDWIN):
            xt = xp.tile([P, P], f32, tag="xt")
            nc.sync.dma_start(xt[:], t_xT[:, w * P:(w + 1) * P])
            ph = psd.tile([P, HID_C], f32, tag="pacc")
            nc.tensor.matmul(ph[:], lhsT=xt[:], rhs=W1_t[:], start=True,
                             stop=True)
            h1t = hp.tile([P, HID_C], f16, tag="h1t")
            nc.scalar.activation(h1t[:], ph[:],
                                 mybir.ActivationFunctionType.Copy,
                                 scale=dinv_t[:, w:w + 1])
            nc.sync.dma_start(cc1_in[w * P:(w + 1) * P, :], h1t[:])

        tc.strict_bb_all_engine_barrier()
        nc.gpsimd.collective_compute(
            "AllGather", mybir.AluOpType.bypass,
            replica_groups=[list(range(NCORE))],
            ins=[cc1_in[:]], outs=[cc1_out[:]],
        )
        tc.strict_bb_all_engine_barrier()

        # ---- phase 2: layer-1 gather + aggregate ----
        out1T = const.tile([HID_C, SLICE], f16)
        qn = 0
        for w in range(NWIN):
            nt = int(tiles_per_win[w])
            wt = win.tile([P, nt, HID_C], f16, tag="wt")
            for b in range(NBANK):
                ln = int(call_len[w, b])
                off = int(call_off[w, b])
                blk0 = (off - int(call_off[w, 0])) // 128
                nc.gpsimd.dma_gather(
                    out_ap=wt[:, blk0:blk0 + ln // 128, :],
                    in_ap=cc1_out[b * BANK:(b + 1) * BANK, :],
                    idxs_ap=aggidx_t[:, off // 16:(off + ln) // 16],
                    num_idxs=ln, num_idxs_reg=ln, elem_size=HID_C,
                    single_packet=False, queue_num=qn % 4,
                )
                qn += 1
            pw = psum.tile([HID_C, WAGG], f32, tag="aggacc")
            tbase = int(call_off[w, 0]) // 128
            for t in range(nt):
                D = dp.tile([P, WAGG], f16, tag="D")
                nc.vector.tensor_scalar(
                    out=D[:], in0=iota_t[:],
                    scalar1=aggdst_t[:, tbase + t:tbase + t + 1], scalar2=None,
                    op0=mybir.AluOpType.is_equal,
                )
                nc.tensor.matmul(pw[:], lhsT=wt[:, t, :], rhs=D[:],
                                 start=(t == 0), stop=(t == nt - 1))
            nc.scalar.activation(out1T[:, w * WAGG:(w + 1) * WAGG], pw[:],
                                 mybir.ActivationFunctionType.Relu,
                                 bias=b1_t[:])

        # ---- phase 3: h2 = (out1 @ W2) * dinv -> zero-padded fp16 table ----
        for w in range(NDWIN):
            ph = psd.tile([P, OUT_C], f32, tag="pacc")
            nc.tensor.matmul(ph[:], lhsT=out1T[:, w * P:(w + 1) * P],
                             rhs=W2_t[:], start=True, stop=True)
            h2t = hp.tile([P, P], f16, tag="h2t")
            nc.vector.memset(h2t[:, OUT_C:P], 0.0)
            nc.scalar.activation(h2t[:, 0:OUT_C], ph[:],
                                 mybir.ActivationFunctionType.Copy,
                                 scale=dinv_t[:, w:w + 1])
            nc.sync.dma_start(cc2_in[w * P:(w + 1) * P, :], h2t[:])

        tc.strict_bb_all_engine_barrier()
        nc.gpsimd.collective_compute(
            "AllGather", mybir.AluOpType.bypass,
            replica_groups=[list(range(NCORE))],
            ins=[cc2_in[:]], outs=[cc2_out[:]],
        )
        tc.strict_bb_all_engine_barrier()

        # ---- phase 4: layer-2 gather + aggregate ----
        for w in range(NWIN):
            nt = int(tiles_per_win[w])
            wt = win.tile([P, nt, P], f16, tag="wt")
            for b in range(NBANK):
                ln = int(call_len[w, b])
                off = int(call_off[w, b])
                blk0 = (off - int(call_off[w, 0])) // 128
                nc.gpsimd.dma_gather(
                    out_ap=wt[:, blk0:blk0 + ln // 128, :],
                    in_ap=cc2_out[b * BANK:(b + 1) * BANK, :],
                    idxs_ap=aggidx_t[:, off // 16:(off + ln) // 16],
                    num_idxs=ln, num_idxs_reg=ln, elem_size=P,
                    single_packet=False, queue_num=qn % 4,
                )
                qn += 1
            pw = psum.tile([P, WAGG], f32, tag="aggacc")
            tbase = int(call_off[w, 0]) // 128
            for t in range(nt):
                D = dp.tile([P, WAGG], f16, tag="D")
                nc.vector.tensor_scalar(
                    out=D[:], in0=iota_t[:],
                    scalar1=aggdst_t[:, tbase + t:tbase + t + 1], scalar2=None,
                    op0=mybir.AluOpType.is_equal,
                )
                nc.tensor.matmul(pw[:], lhsT=wt[:, t, :], rhs=D[:],
                                 start=(t == 0), stop=(t == nt - 1))
            o2 = ev.tile([OUT_C, WAGG], f32, tag="o2")
            nc.scalar.activation(o2[:], pw[0:OUT_C, :],
                                 mybir.ActivationFunctionType.Identity,
                                 bias=b2_t[:])
            nc.sync.dma_start(t_out[:, w * WAGG:(w + 1) * WAGG], o2[:])

    nc.compile()
    return nc


def kernel(x, edge_index, W1, b1, W2, b2):
    global LAST_EXEC_NS, LAST_SCOPES
    x = np.asarray(x, dtype=np.float32)
    edge_index = np.asarray(edge_index)
    W1 = np.asarray(W1, dtype=np.float32)
    b1 = np.asarray(b1, dtype=np.float32)
    W2 = np.asarray(W2, dtype=np.float32)
    b2 = np.asarray(b2, dtype=np.float32)
    src, dst = edge_index[0], edge_index[1]

    sched, agg_idx16, agg_dstrel, deg_srcrel = _build_structure(src, dst)
    nc = _build_bass(sched)

    xT = np.zeros((P, NPAD), dtype=np.float32)
    xT[:, :N] = x.T
    iota = np.broadcast_to(np.arange(WAGG, dtype=np.float16), (P, WAGG)).copy()
    b1c = np.ascontiguousarray(b1.reshape(P, 1).astype(np.float32))
    b2c = np.ascontiguousarray(b2.reshape(OUT_C, 1).astype(np.float32))
    W2h = np.ascontiguousarray(W2.astype(np.float16))

    in_maps = []
    for k in range(NCORE):
        in_maps.append({
            "xT": np.ascontiguousarray(xT[:, k * SLICE:(k + 1) * SLICE]),
            "W1": np.ascontiguousarray(W1),
            "W2h": W2h,
            "b1c": b1c,
            "b2c": b2c,
            "aggidx": np.ascontiguousarray(agg_idx16[k]),
            "aggdst": np.ascontiguousarray(agg_dstrel[k]),
            "degsrc": np.ascontiguousarray(deg_srcrel[k]),
            "iotaf": iota,
        })

    res = run_bass_kernel_spmd(nc, in_maps, core_ids=list(range(NCORE)),
                               trace=TRACE)
    LAST_EXEC_NS = res.exec_time_ns
    LAST_SCOPES = res.per_core_scope_times

    o2T = np.concatenate([res.results[k]["o2T"] for k in range(NCORE)], axis=1)
    return np.ascontiguousarray(o2T.T[:N]).astype(np.float32)


# revision 9
# speedup vs baseline: 1.1029x; 1.0195x over previous
"""GCN encoder (2-layer, out-degree normalized) on 8 Trainium2 NeuronCores.

Strategy: shard dst nodes across cores (12544/core). Host does index-only prep:
sort edges by (dst-window, src-bank) for aggregation, by src-window for degree.
Device: deg via one-hot matmul over src-sorted tiles; h1=(x@W1)*dinv per slice;
AllGather fp16 h1 table; dma_gather (4 swdge queues) fetches per-edge messages;
one-hot compare (vector) + PE matmul accumulate per 256-node dst window;
relu+bias on PSUM evict; layer 2 same with zero-padded fp16 h2 table; out2T
slices assembled and transposed on host.
"""
import numpy as np
from contextlib import ExitStack

import concourse.bass as bass
import concourse.tile as tile
from concourse import bacc, mybir, library_config
from concourse.bass_utils import run_bass_kernel_spmd

P = 128
N = 100000
E = 1600000
IN_C, HID_C, OUT_C = 128, 128, 64
NCORE = 8
NPAD = 100352            # 8 * 12544
SLICE = NPAD // NCORE    # 12544
WAGG = 256               # agg dst-window (nodes)
NWIN = SLICE // WAGG     # 49 agg windows per core
WDEG = 128               # deg src-window
NDWIN = SLICE // WDEG    # 98 deg windows per core
NBANK = 4
BANK = NPAD // NBANK     # 25088 (< 32768, int16-safe)

TRACE = False            # test.py sets True for profiling
LAST_EXEC_NS = None
LAST_SCOPES = None


def _roundup(a, m):
    return (a + m - 1) // m * m


def _wrap16(flat_idx):
    """dma_gather idx layout: [128, n/16], wrapped by 16, replicated 8x."""
    n = flat_idx.shape[0]
    assert n % 16 == 0
    blk = flat_idx.reshape(n // 16, 16).T.astype(np.int16)   # [16, n//16]
    return np.tile(blk, (8, 1))                              # [128, n//16]


def _build_structure(src, dst):
    """Host index prep. Returns (schedule, per-core metadata arrays)."""
    src = src.astype(np.int64)
    dst = dst.astype(np.int64)

    # ---- aggregation: group edges by (global dst-window, src bank) ----
    wglobal = dst // WAGG                    # [E] in [0, 392)
    bank = src // BANK                       # [E] in [0, 4)
    segkey = wglobal * NBANK + bank          # [E]
    order = np.argsort(segkey, kind="stable")
    seg_counts = np.bincount(segkey, minlength=(NPAD // WAGG) * NBANK)
    seg_counts = seg_counts.reshape(NCORE, NWIN, NBANK)
    # uniform schedule: per (window,bank) call length = max over cores, 128-mult
    call_len = _roundup(seg_counts.max(axis=0), 128)         # [NWIN, NBANK]
    seg_starts = np.zeros((NCORE, NWIN, NBANK), dtype=np.int64)
    cum = np.cumsum(np.bincount(segkey, minlength=(NPAD // WAGG) * NBANK))
    seg_starts_flat = cum - np.bincount(segkey, minlength=(NPAD // WAGG) * NBANK)
    seg_starts = seg_starts_flat.reshape(NCORE, NWIN, NBANK)

    slots_per_win = call_len.sum(axis=1)                     # [NWIN]
    tiles_per_win = slots_per_win // 128                     # [NWIN]
    total_slots = int(slots_per_win.sum())
    total_tiles = total_slots // 128

    # slot offsets of each (w, b) call within the per-core slot space
    call_off = np.zeros((NWIN, NBANK), dtype=np.int64)
    cur = 0
    for w in range(NWIN):
        for b in range(NBANK):
            call_off[w, b] = cur
            cur += call_len[w, b]

    agg_idx = np.zeros((NCORE, total_slots), dtype=np.int16)
    agg_dst = np.full((NCORE, total_slots), -1.0, dtype=np.float32)
    src_sorted = src[order]
    dst_sorted = dst[order]
    for k in range(NCORE):
        for w in range(NWIN):
            for b in range(NBANK):
                cnt = seg_counts[k, w, b]
                s0 = seg_starts[k, w, b]
                o0 = call_off[w, b]
                agg_idx[k, o0:o0 + cnt] = (src_sorted[s0:s0 + cnt] - b * BANK)
                agg_dst[k, o0:o0 + cnt] = (
                    dst_sorted[s0:s0 + cnt] - (k * SLICE + w * WAGG)
                ).astype(np.float32)

    # idx tensor [128, total_slots//16]; dstrel tensor [128, total_tiles]
    agg_idx16 = np.stack([_wrap16(agg_idx[k]) for k in range(NCORE)])
    agg_dstrel = np.stack(
        [agg_dst[k].reshape(total_tiles, 128).T for k in range(NCORE)]
    )  # [NCORE, 128, total_tiles]

    # ---- degree: group edges by global src-window of 128 ----
    dwin = src // WDEG                       # [E] in [0, 784)
    dorder = np.argsort(dwin, kind="stable")
    dcounts = np.bincount(dwin, minlength=NPAD // WDEG).reshape(NCORE, NDWIN)
    dlen = _roundup(dcounts.max(axis=0), 128)                # [NDWIN]
    dtiles_per_win = dlen // 128
    dtotal_slots = int(dlen.sum())
    dtotal_tiles = dtotal_slots // 128
    doff = np.concatenate([[0], np.cumsum(dlen)[:-1]])
    dstarts_flat = np.concatenate(
        [[0], np.cumsum(np.bincount(dwin, minlength=NPAD // WDEG))[:-1]]
    ).reshape(NCORE, NDWIN)
    src_dsorted = src[dorder]
    deg_src = np.full((NCORE, dtotal_slots), -1.0, dtype=np.float32)
    for k in range(NCORE):
        for w in range(NDWIN):
            cnt = dcounts[k, w]
            s0 = dstarts_flat[k, w]
            o0 = doff[w]
            deg_src[k, o0:o0 + cnt] = (
                src_dsorted[s0:s0 + cnt] - (k * SLICE + w * WDEG)
            ).astype(np.float32)
    deg_srcrel = np.stack(
        [deg_src[k].reshape(dtotal_tiles, 128).T for k in range(NCORE)]
    )

    sched = {
        "call_len": call_len, "call_off": call_off,
        "tiles_per_win": tiles_per_win, "total_slots": total_slots,
        "total_tiles": total_tiles,
        "dlen": dlen, "dtiles_per_win": dtiles_per_win,
        "dtotal_tiles": dtotal_tiles,
    }
    return sched, agg_idx16, agg_dstrel, deg_srcrel


def _build_bass(sched):
    call_len = sched["call_len"]
    call_off = sched["call_off"]
    tiles_per_win = sched["tiles_per_win"]
    total_slots = sched["total_slots"]
    total_tiles = sched["total_tiles"]
    dtiles_per_win = sched["dtiles_per_win"]
    dtotal_tiles = sched["dtotal_tiles"]

    f32, f16, i16 = mybir.dt.float32, mybir.dt.float16, mybir.dt.int16
    nc = bacc.Bacc("TRN2", target_bir_lowering=False, debug=False,
                   num_devices=NCORE, num_swdge_queues=4)

    t_xT = nc.dram_tensor("xT", [P, SLICE], f32, kind="ExternalInput")
    t_W1 = nc.dram_tensor("W1", [IN_C, HID_C], f32, kind="ExternalInput")
    t_W2 = nc.dram_tensor("W2h", [HID_C, OUT_C], f16, kind="ExternalInput")
    t_b1 = nc.dram_tensor("b1c", [P, 1], f32, kind="ExternalInput")
    t_b2 = nc.dram_tensor("b2c", [OUT_C, 1], f32, kind="ExternalInput")
    t_aggidx = nc.dram_tensor("aggidx", [P, total_slots // 16], i16,
                              kind="ExternalInput")
    t_aggdst = nc.dram_tensor("aggdst", [P, total_tiles], f32,
                              kind="ExternalInput")
    t_degsrc = nc.dram_tensor("degsrc", [P, dtotal_tiles], f32,
                              kind="ExternalInput")
    t_iota = nc.dram_tensor("iotaf", [P, WAGG], f16, kind="ExternalInput")

    t_out = nc.dram_tensor("o2T", [OUT_C, SLICE], f32, kind="ExternalOutput")

    cc1_in = nc.dram_tensor("cc1_in", [SLICE, HID_C], f16, kind="Internal")
    cc1_out = nc.dram_tensor("cc1_out", [NPAD, HID_C], f16, kind="Internal",
                             addr_space="Shared")
    cc2_in = nc.dram_tensor("cc2_in", [SLICE, P], f16, kind="Internal")
    cc2_out = nc.dram_tensor("cc2_out", [NPAD, P], f16, kind="Internal",
                             addr_space="Shared")

    with tile.TileContext(nc) as tc, ExitStack() as ctx:
        const = ctx.enter_context(tc.tile_pool(name="const", bufs=1))
        meta = ctx.enter_context(tc.tile_pool(name="meta", bufs=1))
        xp = ctx.enter_context(tc.tile_pool(name="xp", bufs=4))
        hp = ctx.enter_context(tc.tile_pool(name="hp", bufs=4))
        win = ctx.enter_context(tc.tile_pool(name="win", bufs=4))
        dp = ctx.enter_context(tc.tile_pool(name="dp", bufs=12))
        ev = ctx.enter_context(tc.tile_pool(name="ev", bufs=4))
        psum = ctx.enter_context(tc.tile_pool(name="psum", bufs=4, space="PSUM"))
        psd = ctx.enter_context(tc.tile_pool(name="psd", bufs=3, space="PSUM"))

        nc.gpsimd.load_library(library_config.mlp)

        # constants / metadata loads
        W1_t = const.tile([IN_C, HID_C], f32)
        nc.sync.dma_start(W1_t[:], t_W1[:])
        W2_t = const.tile([HID_C, OUT_C], f16)
        nc.sync.dma_start(W2_t[:], t_W2[:])
        b1_t = const.tile([P, 1], f32)
        nc.sync.dma_start(b1_t[:], t_b1[:])
        b2_t = const.tile([OUT_C, 1], f32)
        nc.sync.dma_start(b2_t[:], t_b2[:])
        iota_t = const.tile([P, WAGG], f16)
        nc.sync.dma_start(iota_t[:], t_iota[:])
        ones_t = const.tile([P, 1], f16)
        nc.vector.memset(ones_t[:], 1.0)

        aggidx_t = meta.tile([P, total_slots // 16], i16)
        nc.sync.dma_start(aggidx_t[:], t_aggidx[:])
        aggdst_t = meta.tile([P, total_tiles], f32)
        nc.sync.dma_start(aggdst_t[:], t_aggdst[:])
        degsrc_t = meta.tile([P, dtotal_tiles], f32)
        nc.sync.dma_start(degsrc_t[:], t_degsrc[:])

        # ---- phase 0: degree (one-hot matmul over src-sorted tiles) ----
        # ones as stationary (loaded once) -> deg lands as rows [1, 128];
        # transpose back to per-partition columns with tiny matmuls.
        deg_row = const.tile([1, NDWIN * WDEG], f32)
        ones1_t = const.tile([1, 1], f32)
        nc.vector.memset(ones1_t[:], 1.0)
        deg_t = const.tile([P, NDWIN], f32)
        dt_idx = 0
        for w in range(NDWIN):
            pt = psd.tile([1, WDEG], f32, tag="pacc")
            nt = dtiles_per_win[w]
            for t in range(nt):
                S = dp.tile([P, WDEG], f16, tag="S")
                nc.vector.tensor_scalar(
                    out=S[:], in0=iota_t[:, 0:WDEG],
                    scalar1=degsrc_t[:, dt_idx:dt_idx + 1], scalar2=None,
                    op0=mybir.AluOpType.is_equal,
                )
                nc.tensor.matmul(pt[:], lhsT=ones_t[:], rhs=S[:],
                                 start=(t == 0), stop=(t == nt - 1))
                dt_idx += 1
            nc.scalar.copy(deg_row[:, w * WDEG:(w + 1) * WDEG], pt[:])
        for w in range(NDWIN):
            ptt = psd.tile([P, 1], f32, tag="pacc")
            nc.tensor.matmul(ptt[:], lhsT=deg_row[:, w * WDEG:(w + 1) * WDEG],
                             rhs=ones1_t[:], start=True, stop=True)
            nc.scalar.copy(deg_t[:, w:w + 1], ptt[:])
        dinv_t = const.tile([P, NDWIN], f32)
        nc.vector.tensor_scalar_max(dinv_t[:], deg_t[:], 1.0)
        nc.vector.reciprocal(dinv_t[:], dinv_t[:])

        # ---- phase 1: h1 = (x @ W1) * dinv, per 128-node block ----
        for w in range(NDWIN):
            xt = xp.tile([P, P], f32, tag="xt")
            nc.sync.dma_start(xt[:], t_xT[:, w * P:(w + 1) * P])
            ph = psd.tile([P, HID_C], f32, tag="pacc")
            nc.tensor.matmul(ph[:], lhsT=xt[:], rhs=W1_t[:], start=True,
                             stop=True)
            h1t = hp.tile([P, HID_C], f16, tag="h1t")
            nc.scalar.activation(h1t[:], ph[:],
                                 mybir.ActivationFunctionType.Copy,
                                 scale=dinv_t[:, w:w + 1])
            nc.sync.dma_start(cc1_in[w * P:(w + 1) * P, :], h1t[:])

        nc.gpsimd.collective_compute(
            "AllGather", mybir.AluOpType.bypass,
            replica_groups=[list(range(NCORE))],
            ins=[cc1_in[:]], outs=[cc1_out[:]],
        )

        # ---- phase 2: layer-1 gather + aggregate ----
        out1T = const.tile([HID_C, SLICE], f16)
        qn = 0
        for w in range(NWIN):
            nt = int(tiles_per_win[w])
            wt = win.tile([P, nt, HID_C], f16, tag="wt")
            for b in range(NBANK):
                ln = int(call_len[w, b])
                off = int(call_off[w, b])
                blk0 = (off - int(call_off[w, 0])) // 128
                nc.gpsimd.dma_gather(
                    out_ap=wt[:, blk0:blk0 + ln // 128, :],
                    in_ap=cc1_out[b * BANK:(b + 1) * BANK, :],
                    idxs_ap=aggidx_t[:, off // 16:(off + ln) // 16],
                    num_idxs=ln, num_idxs_reg=ln, elem_size=HID_C,
                    single_packet=False, queue_num=qn % 4,
                )
                qn += 1
            pw = psum.tile([HID_C, WAGG], f32, tag="aggacc")
            tbase = int(call_off[w, 0]) // 128
            for t in range(nt):
                D = dp.tile([P, WAGG], f16, tag="D")
                nc.vector.tensor_scalar(
                    out=D[:], in0=iota_t[:],
                    scalar1=aggdst_t[:, tbase + t:tbase + t + 1], scalar2=None,
                    op0=mybir.AluOpType.is_equal,
                )
                nc.tensor.matmul(pw[:], lhsT=wt[:, t, :], rhs=D[:],
                                 start=(t == 0), stop=(t == nt - 1))
            nc.scalar.activation(out1T[:, w * WAGG:(w + 1) * WAGG], pw[:],
                                 mybir.ActivationFunctionType.Relu,
                                 bias=b1_t[:])

        # ---- phase 3: h2 = (out1 @ W2) * dinv -> zero-padded fp16 table ----
        for w in range(NDWIN):
            ph = psd.tile([P, OUT_C], f32, tag="pacc")
            nc.tensor.matmul(ph[:], lhsT=out1T[:, w * P:(w + 1) * P],
                             rhs=W2_t[:], start=True, stop=True)
            h2t = hp.tile([P, P], f16, tag="h2t")
            nc.vector.memset(h2t[:, OUT_C:P], 0.0)
            nc.scalar.activation(h2t[:, 0:OUT_C], ph[:],
                                 mybir.ActivationFunctionType.Copy,
                                 scale=dinv_t[:, w:w + 1])
            nc.sync.dma_start(cc2_in[w * P:(w + 1) * P, :], h2t[:])

        nc.gpsimd.collective_compute(
            "AllGather", mybir.AluOpType.bypass,
            replica_groups=[list(range(NCORE))],
            ins=[cc2_in[:]], outs=[cc2_out[:]],
        )

        # ---- phase 4: layer-2 gather + aggregate ----
        for w in range(NWIN):
            nt = int(tiles_per_win[w])
            wt = win.tile([P, nt, P], f16, tag="wt")
            for b in range(NBANK):
                ln = int(call_len[w, b])
                off = int(call_off[w, b])
                blk0 = (off - int(call_off[w, 0])) // 128
                nc.gpsimd.dma_gather(
                    out_ap=wt[:, blk0:blk0 + ln // 128, :],
                    in_ap=cc2_out[b * BANK:(b + 1) * BANK, :],
                    idxs_ap=aggidx_t[:, off // 16:(off + ln) // 16],
                    num_idxs=ln, num_idxs_reg=ln, elem_size=P,
                    single_packet=False, queue_num=qn % 4,
                )
                qn += 1
            pw = psum.tile([P, WAGG], f32, tag="aggacc")
            tbase = int(call_off[w, 0]) // 128
            for t in range(nt):
                D = dp.tile([P, WAGG], f16, tag="D")
                nc.vector.tensor_scalar(
                    out=D[:], in0=iota_t[:],
                    scalar1=aggdst_t[:, tbase + t:tbase + t + 1], scalar2=None,
                    op0=mybir.AluOpType.is_equal,
                )
                nc.tensor.matmul(pw[:], lhsT=wt[:, t, :], rhs=D[:],
                                 start=(t == 0), stop=(t == nt - 1))
            o2 = ev.tile([OUT_C, WAGG], f32, tag="o2")
            nc.scalar.activation(o2[:], pw[0:OUT_C, :],
                                 mybir.ActivationFunctionType.Identity,
                                 bias=b2_t[:])
            nc.sync.dma_start(t_out[:, w * WAGG:(w + 1) * WAGG], o2[:])

    nc.compile()
    return nc


def kernel(x, edge_index, W1, b1, W2, b2):
    global LAST_EXEC_NS, LAST_SCOPES
    x = np.asarray(x, dtype=np.float32)
    edge_index = np.asarray(edge_index)
    W1 = np.asarray(W1, dtype=np.float32)
    b1 = np.asarray(b1, dtype=np.float32)
    W2 = np.asarray(W2, dtype=np.float32)
    b2 = np.asarray(b2, dtype=np.float32)
    src, dst = edge_index[0], edge_index[1]

    sched, agg_idx16, agg_dstrel, deg_srcrel = _build_structure(src, dst)
    nc = _build_bass(sched)

    xT = np.zeros((P, NPAD), dtype=np.float32)
    xT[:, :N] = x.T
    iota = np.broadcast_to(np.arange(WAGG, dtype=np.float16), (P, WAGG)).copy()
    b1c = np.ascontiguousarray(b1.reshape(P, 1).astype(np.float32))
    b2c = np.ascontiguousarray(b2.reshape(OUT_C, 1).astype(np.float32))
    W2h = np.ascontiguousarray(W2.astype(np.float16))

    in_maps = []
    for k in range(NCORE):
        in_maps.append({
            "xT": np.ascontiguousarray(xT[:, k * SLICE:(k + 1) * SLICE]),
            "W1": np.ascontiguousarray(W1),
            "W2h": W2h,
            "b1c": b1c,
            "b2c": b2c,
            "aggidx": np.ascontiguousarray(agg_idx16[k]),
            "aggdst": np.ascontiguousarray(agg_dstrel[k]),
            "degsrc": np.ascontiguousarray(deg_srcrel[k]),
            "iotaf": iota,
        })

    res = run_bass_kernel_spmd(nc, in_maps, core_ids=list(range(NCORE)),
                               trace=TRACE)
    LAST_EXEC_NS = res.exec_time_ns
    LAST_SCOPES = res.per_core_scope_times

    o2T = np.concatenate([res.results[k]["o2T"] for k in range(NCORE)], axis=1)
    return np.ascontiguousarray(o2T.T[:N]).astype(np.float32)
